# revision 1
# baseline (speedup 1.0000x reference)
"""Trainium2 Bass kernel for nn_CausalWanModel (frame-block-causal attention).

Self-contained: hardcodes shapes from the problem spec.
  B=1, T=3120, D=1536, H=12 heads, hd=128, frame_seqlen=780, 8 cores.

Sharding: sequence-parallel. Core c owns tokens [390c, 390c+390) for
q/k/v projections, attention (its queries vs all keys, block-causal mask
via per-key additive bias on the exp), and the o-projection rows. k/v are
exchanged with a single AllGather; the host gathers the 8 row-slices.

Matmuls run in bf16 (fp32 PSUM accumulation); RMSNorm statistics in fp32.
"""

import math

import numpy as np
import ml_dtypes

import concourse.bacc as bacc
import concourse.mybir as mybir
import concourse.tile as tile
from concourse.bass_utils import run_bass_kernel_spmd

F32 = mybir.dt.float32
BF16 = mybir.dt.bfloat16

NC = 8
T = 3120
D = 1536
H = 12
HD = 128
L = 780  # frame_seqlen
CHUNK = T // NC  # 390 tokens per core
KC = D // 128  # 12 contraction chunks
EPS = 1e-6
SCALE = 1.0 / math.sqrt(HD)

PADT = 512  # per-rank padded token count in the gathered v / padded key grid
KEY_TILES = []
for _r in range(NC):
    KEY_TILES += [(_r * PADT + 0, 128), (_r * PADT + 128, 128),
                  (_r * PADT + 256, 128), (_r * PADT + 384, 6)]
TOK_SUBS = [(0, 128), (128, 128), (256, 128), (384, 6)]
COLG = [(g * 512, 512) for g in range(3)]

K_ELEMS = D * CHUNK
V_ELEMS = CHUNK * D


def build_kernel(apply_bias_qk=False, apply_g=False, apply_bias_v=False,
                 apply_bias_o=False, debug=False):
    nc = bacc.Bacc("TRN2", target_bir_lowering=False, debug=False, num_devices=NC)

    # ---- I/O ----
    xT = nc.dram_tensor("xT", [D, CHUNK], BF16, kind="ExternalInput")
    # weights pre-tiled on host: wq/wk[d] = [128, KC*128] (lhsT chunks),
    # wv/wo[g] = [128, KC*512] (rhs chunks per column group)
    wq = nc.dram_tensor("wq", [KC, 128, KC * 128], BF16, kind="ExternalInput")
    wk = nc.dram_tensor("wk", [KC, 128, KC * 128], BF16, kind="ExternalInput")
    wv = nc.dram_tensor("wv", [3, 128, KC * 512], BF16, kind="ExternalInput")
    wo = nc.dram_tensor("wo", [3, 128, KC * 512], BF16, kind="ExternalInput")
    cost = nc.dram_tensor("cost", [128, CHUNK], F32, kind="ExternalInput")
    sint = nc.dram_tensor("sint", [128, CHUNK], F32, kind="ExternalInput")
    maskv = nc.dram_tensor("maskv", [128, NC], F32, kind="ExternalInput")
    bqk2 = nc.dram_tensor("bqk2", [2 * KC, 128], F32, kind="ExternalInput")
    gqk2 = nc.dram_tensor("gqk2", [2 * KC, 128], F32, kind="ExternalInput")
    bvo = nc.dram_tensor("bvo", [2, D], F32, kind="ExternalInput")
    out_part = nc.dram_tensor("out_part", [CHUNK, D], F32, kind="ExternalOutput")

    # ---- collective buffers ----
    k_in = nc.dram_tensor("k_in", [K_ELEMS], BF16)
    v_in = nc.dram_tensor("v_in", [PADT * D], BF16)
    k_out = nc.dram_tensor("k_out", [NC, K_ELEMS], BF16, addr_space="Shared")
    v_out = nc.dram_tensor("v_out", [NC, PADT * D], BF16, addr_space="Shared")

    if debug:
        dbg_qT = nc.dram_tensor("dbg_qT", [128, KC * CHUNK], F32, kind="ExternalOutput")
        dbg_kT = nc.dram_tensor("dbg_kT", [128, KC * CHUNK], F32, kind="ExternalOutput")
        dbg_sums = nc.dram_tensor("dbg_sums", [H, CHUNK], F32, kind="ExternalOutput")
        dbg_attnT = nc.dram_tensor("dbg_attnT", [128, KC * CHUNK], F32,
                                   kind="ExternalOutput")

    kT_view = k_in.ap().rearrange("(r t) -> r t", t=CHUNK)
    v_view = v_in.ap().rearrange("(t c) -> t c", c=D)

    with tile.TileContext(nc) as tc:
        with tc.tile_pool(name="const", bufs=1) as cpool:
            xT_sb = cpool.tile([128, KC * CHUNK], BF16, tag="xT_sb")
            qT_sb = cpool.tile([128, KC * CHUNK], BF16, tag="qT_sb")
            attnT_sb = cpool.tile([128, KC * CHUNK], BF16, tag="attnT_sb")
            cost_sb = cpool.tile([128, CHUNK], F32, tag="cost_sb")
            sint_sb = cpool.tile([128, CHUNK], F32, tag="sint_sb")
            masks_sb = cpool.tile([128, NC], F32, tag="masks_sb")
            ones_f32 = cpool.tile([128, 1], F32, tag="ones_f32")
            ones_bf = cpool.tile([128, 1], BF16, tag="ones_bf")
            sq_scale = cpool.tile([1, CHUNK], F32, tag="sq_scale")
            sk_scale = cpool.tile([1, CHUNK], F32, tag="sk_scale")
            sq_bc = cpool.tile([128, CHUNK], F32, tag="sq_bc")
            sk_bc = cpool.tile([128, CHUNK], F32, tag="sk_bc")

            eps_sb = cpool.tile([1, 1], F32, tag="eps_sb")
            nc.gpsimd.memset(ones_f32[:, :], 1.0)
            nc.gpsimd.memset(ones_bf[:, :], 1.0)
            nc.gpsimd.memset(eps_sb[:, :], EPS)

            for d in range(KC):
                nc.scalar.dma_start(out=xT_sb[:, d * CHUNK:(d + 1) * CHUNK],
                                    in_=xT[d * 128:(d + 1) * 128, :])
            nc.sync.dma_start(out=cost_sb[:, :], in_=cost[:, :])
            nc.sync.dma_start(out=sint_sb[:, :], in_=sint[:, :])
            nc.sync.dma_start(out=masks_sb[:, :], in_=maskv[:, :])
            bqk_sb = gqk_sb = bvo_sb = None
            if apply_bias_qk:
                bqk_sb = cpool.tile([128, 2 * KC], F32, tag="bqk_sb")
                nc.sync.dma_start(out=bqk_sb[:, :],
                                  in_=bqk2.ap().rearrange("c p -> p c"))
            if apply_g:
                gqk_sb = cpool.tile([128, 2 * KC], F32, tag="gqk_sb")
                nc.sync.dma_start(out=gqk_sb[:, :],
                                  in_=gqk2.ap().rearrange("c p -> p c"))
            if apply_bias_v or apply_bias_o:
                bvo_sb = cpool.tile([2, D], F32, tag="bvo_sb")
                nc.sync.dma_start(out=bvo_sb[:, :], in_=bvo[:, :])

            # ===== Phase 1: projections + rmsnorm + rope (k -> AG_k -> v -> AG_v -> q) =====
            with tc.tile_pool(name="p1sb", bufs=3) as p1sb, \
                 tc.tile_pool(name="p1w", bufs=5) as p1w, \
                 tc.tile_pool(name="upool", bufs=1) as upool, \
                 tc.tile_pool(name="p1ps", bufs=3, space="PSUM") as p1ps, \
                 tc.tile_pool(name="ssqps", bufs=1, space="PSUM") as ssqps:

                u_tiles = {(name, d): upool.tile([128, CHUNK], F32,
                                                 name=f"u_{name}_{d}",
                                                 tag=f"u_{name}_{d}")
                           for name in ("q", "k") for d in range(KC)}
                ssq_ps = {}

                def qk_proj(name, w, is_q):
                    ssq_ps[name] = ssqps.tile([1, CHUNK], F32, name=f"ssq_{name}",
                                              tag=f"ssq_{name}")
                    for d in range(KC):
                        wt = p1w.tile([128, D], BF16, tag="wqk_t", name="wqk_t")
                        nc.sync.dma_start(out=wt[:, :], in_=w[d, :, :])
                        ps = p1ps.tile([128, CHUNK], F32, tag="proj_ps",
                                       name="proj_ps")
                        for c in range(KC):
                            nc.tensor.matmul(
                                ps[:, :],
                                wt[:, c * 128:(c + 1) * 128],
                                xT_sb[:, c * CHUNK:(c + 1) * CHUNK],
                                start=(c == 0), stop=(c == KC - 1))
                        ur = u_tiles[(name, d)]
                        if apply_bias_qk:
                            bias_col = (0 if is_q else KC) + d
                            nc.vector.tensor_scalar_add(
                                ur[:, :], ps[:, :], bqk_sb[:, bias_col:bias_col + 1])
                        else:
                            nc.scalar.copy(ur[:, :], ps[:, :])
                        sq = p1sb.tile([128, CHUNK], BF16, tag="sqsb", name="sqsb")
                        nc.vector.tensor_tensor(sq[:, :], ur[:, :], ur[:, :],
                                                mybir.AluOpType.mult)
                        nc.tensor.matmul(ssq_ps[name][:, :], ones_bf[:, :], sq[:, :],
                                         start=(d == 0), stop=(d == KC - 1))

                def qk_scales(name, stile, sbc):
                    nc.scalar.activation(stile[:, :], ssq_ps[name][:, :],
                                         mybir.ActivationFunctionType.Sqrt,
                                         bias=eps_sb[:, :], scale=1.0 / D)
                    nc.vector.reciprocal(stile[:, :], stile[:, :])
                    nc.gpsimd.partition_broadcast(sbc[:, :], stile[:, :])

                def qk_rope(name, sbc):
                    for d in range(KC):
                        ur = u_tiles[(name, d)]
                        qs = p1sb.tile([128, CHUNK], F32, tag="qs", name="qs")
                        nc.vector.tensor_tensor(
                            qs[:, :], ur[:, :], sbc[:, :],
                            mybir.AluOpType.mult)
                        if apply_g:
                            gcol = (0 if name == "q" else KC) + d
                            nc.vector.tensor_scalar_mul(
                                qs[:, :], qs[:, :], gqk_sb[:, gcol:gcol + 1])
                        qsw = p1sb.tile([128, CHUNK], F32, tag="qsw", name="qsw")
                        nc.scalar.dma_start(out=qsw[0:64, :], in_=qs[64:128, :])
                        nc.scalar.dma_start(out=qsw[64:128, :], in_=qs[0:64, :])
                        t1 = p1sb.tile([128, CHUNK], F32, tag="rope_t1", name="rope_t1")
                        t2 = p1sb.tile([128, CHUNK], F32, tag="rope_t2", name="rope_t2")
                        nc.vector.tensor_tensor(t1[:, :], qs[:, :], cost_sb[:, :],
                                                mybir.AluOpType.mult)
                        nc.vector.tensor_tensor(t2[:, :], qsw[:, :], sint_sb[:, :],
                                                mybir.AluOpType.mult)
                        if name == "q":
                            dst = qT_sb[:, d * CHUNK:(d + 1) * CHUNK]
                            nc.vector.tensor_tensor(dst[:, :], t1[:, :], t2[:, :],
                                                    mybir.AluOpType.add)
                            if debug:
                                df = p1sb.tile([128, CHUNK], F32, tag="dbgf",
                                               name="dbgf")
                                nc.vector.tensor_copy(df[:, :], dst)
                                nc.sync.dma_start(
                                    out=dbg_qT[:, d * CHUNK:(d + 1) * CHUNK],
                                    in_=df[:, :])
                        else:
                            kr = p1sb.tile([128, CHUNK], BF16, tag="krope",
                                           name="krope")
                            nc.vector.tensor_tensor(kr[:, :], t1[:, :], t2[:, :],
                                                    mybir.AluOpType.add)
                            nc.sync.dma_start(
                                out=kT_view[d * 128:(d + 1) * 128, :],
                                in_=kr[:, :])
                            if debug:
                                df = p1sb.tile([128, CHUNK], F32, tag="dbgf",
                                               name="dbgf")
                                nc.vector.tensor_copy(df[:, :], kr[:, :])
                                nc.sync.dma_start(
                                    out=dbg_kT[:, d * CHUNK:(d + 1) * CHUNK],
                                    in_=df[:, :])

                # ---- v projection, then AG_v ----
                for gi, (c0, csz) in enumerate(COLG):
                    wt = p1w.tile([128, KC * 512], BF16, tag="wv_t", name="wv_t")
                    nc.sync.dma_start(out=wt[:, :], in_=wv[gi, :, :])
                    for (t0, tsz) in TOK_SUBS:
                        ps = p1ps.tile([128, 512], F32, tag="v_ps", name="v_ps")
                        for c in range(KC):
                            nc.tensor.matmul(
                                ps[0:tsz, :],
                                xT_sb[:, c * CHUNK + t0:c * CHUNK + t0 + tsz],
                                wt[:, c * 512:(c + 1) * 512],
                                start=(c == 0), stop=(c == KC - 1))
                        vsb = p1sb.tile([128, 512], BF16, tag="vsb", name="vsb")
                        if apply_bias_v:
                            bvb = p1sb.tile([128, 512], F32, tag="bvb", name="bvb")
                            nc.gpsimd.partition_broadcast(
                                bvb[:, :], bvo_sb[0:1, c0:c0 + csz])
                            nc.vector.tensor_tensor(
                                vsb[0:tsz, :], ps[0:tsz, :], bvb[0:tsz, :],
                                mybir.AluOpType.add)
                        else:
                            nc.vector.tensor_copy(vsb[0:tsz, :], ps[0:tsz, :])
                        nc.sync.dma_start(out=v_view[t0:t0 + tsz, c0:c0 + csz],
                                          in_=vsb[0:tsz, :])
                nc.gpsimd.collective_compute(
                    "AllGather", mybir.AluOpType.bypass,
                    ins=[v_in.ap().opt()],
                    outs=[v_out.ap().opt()],
                    replica_groups=[list(range(NC))],
                )
                # ---- k first, then AG_k ----
                qk_proj("k", wk, False)
                qk_scales("k", sk_scale, sk_bc)
                qk_rope("k", sk_bc)
                nc.gpsimd.collective_compute(
                    "AllGather", mybir.AluOpType.bypass,
                    ins=[k_in.ap().opt()],
                    outs=[k_out.ap().opt()],
                    replica_groups=[list(range(NC))],
                )

                # ---- q last (overlaps the collectives) ----
                qk_proj("q", wq, True)
                qk_scales("q", sq_scale, sq_bc)
                qk_rope("q", sq_bc)

            # =========== Phase 3: attention ===========
            NKT = len(KEY_TILES)
            with tc.tile_pool(name="p4w", bufs=1) as p4w:
              with tc.tile_pool(name="a_k", bufs=3) as akp, \
                   tc.tile_pool(name="a_v", bufs=2) as avp, \
                   tc.tile_pool(name="a_p", bufs=32) as app, \
                   tc.tile_pool(name="a_sb", bufs=3) as asb, \
                   tc.tile_pool(name="a_ps", bufs=2, space="PSUM") as aps, \
                   tc.tile_pool(name="acc_ps", bufs=3, space="PSUM") as accps, \
                   tc.tile_pool(name="sum_ps", bufs=1, space="PSUM") as sumps:
                # prefetch Wo column groups during attention
                wo_tiles = []
                for gi, (c0, csz) in enumerate(COLG):
                    wt = p4w.tile([128, KC * 512], BF16, tag=f"wo_t{gi}",
                                  name=f"wo_t{gi}")
                    nc.sync.dma_start(out=wt[:, :], in_=wo[gi, :, :])
                    wo_tiles.append(wt)

                for h in range(H):
                    kt_sb = akp.tile([128, NC * PADT], BF16, tag="kt_sb",
                                     name="kt_sb")
                    for r in range(NC):
                        nc.sync.dma_start(
                            out=kt_sb[:, r * PADT:r * PADT + CHUNK],
                            in_=k_out.ap()[r, :]
                            .rearrange("(row t) -> row t", t=CHUNK)
                            [h * 128:(h + 1) * 128, :])
                    vt_sb = avp.tile([128, NKT, 128], BF16, tag="vt_sb",
                                     name="vt_sb")
                    nc.sync.dma_start(
                        out=vt_sb[:, :, :],
                        in_=v_out.ap().rearrange("r (t c) -> (r t) c", c=D)
                        [:, h * 128:(h + 1) * 128]
                        .rearrange("(t p) c -> p t c", p=128))

                    acc = accps.tile([128, CHUNK], F32, tag="acc", name="acc")
                    sums = sumps.tile([1, CHUNK], F32, tag="sums", name="sums")
                    sumacc = asb.tile([128, CHUNK], F32, tag="sumacc",
                                      name="sumacc")
                    nc.vector.memset(sumacc[:, :], 0.0)
                    for r in range(NC):
                        for j2 in range(2):  # pairs of key tiles
                            sc2 = aps.tile([128, 2, 512], F32, tag="sc2",
                                           name="sc2")
                            pr2 = app.tile([128, 2, CHUNK], BF16, tag="pr2",
                                           name="pr2")
                            for jj in range(2):
                                kt = r * 4 + j2 * 2 + jj
                                k0, ksz = KEY_TILES[kt]
                                nc.tensor.matmul(
                                    sc2[0:ksz, jj, 0:CHUNK],
                                    kt_sb[:, k0:k0 + ksz],
                                    qT_sb[:, h * CHUNK:(h + 1) * CHUNK],
                                    start=True, stop=True)
                            nc.scalar.activation(
                                pr2[:, :, :], sc2[:, :, 0:CHUNK],
                                mybir.ActivationFunctionType.Exp,
                                bias=masks_sb[:, r:r + 1], scale=SCALE)
                            for jj in range(2):
                                kt = r * 4 + j2 * 2 + jj
                                k0, ksz = KEY_TILES[kt]
                                first = (kt == 0)
                                last = (kt == NKT - 1)
                                nc.tensor.matmul(
                                    acc[:, :],
                                    vt_sb[0:ksz, kt, :],
                                    pr2[0:ksz, jj, :],
                                    start=first, stop=last)
                                nc.vector.tensor_tensor(
                                    sumacc[0:ksz, :], sumacc[0:ksz, :],
                                    pr2[0:ksz, jj, :],
                                    mybir.AluOpType.add)
                    nc.tensor.matmul(sums[:, :], ones_f32[:, :], sumacc[:, :],
                                     start=True, stop=True)
                    rec = asb.tile([1, CHUNK], F32, tag="rec", name="rec")
                    nc.vector.reciprocal(rec[:, :], sums[:, :])
                    recb = asb.tile([128, CHUNK], F32, tag="recb", name="recb")
                    nc.gpsimd.partition_broadcast(recb[:, :], rec[:, :])
                    nc.vector.tensor_tensor(
                        attnT_sb[:, h * CHUNK:(h + 1) * CHUNK],
                        acc[:, :], recb[:, :],
                        mybir.AluOpType.mult)
                    if debug:
                        ssb = asb.tile([1, CHUNK], F32, tag="ssb", name="ssb")
                        nc.vector.tensor_copy(ssb[:, :], sums[:, :])
                        nc.sync.dma_start(out=dbg_sums[h:h + 1, :], in_=ssb[:, :])
                        da = asb.tile([128, CHUNK], F32, tag="da", name="da")
                        nc.vector.tensor_copy(
                            da[:, :], attnT_sb[:, h * CHUNK:(h + 1) * CHUNK])
                        nc.sync.dma_start(
                            out=dbg_attnT[:, h * CHUNK:(h + 1) * CHUNK],
                            in_=da[:, :])

            # =========== Phase 4: o-projection ===========
              with tc.tile_pool(name="p4sb", bufs=3) as p4sb, \
                   tc.tile_pool(name="p4ps", bufs=3, space="PSUM") as p4ps:
                  for gi, (c0, csz) in enumerate(COLG):
                      wt = wo_tiles[gi]
                      for (t0, tsz) in TOK_SUBS:
                          ps = p4ps.tile([128, 512], F32, tag="o_ps", name="o_ps")
                          for hh in range(KC):
                              nc.tensor.matmul(
                                  ps[0:tsz, :],
                                  attnT_sb[:, hh * CHUNK + t0:hh * CHUNK + t0 + tsz],
                                  wt[:, hh * 512:(hh + 1) * 512],
                                  start=(hh == 0), stop=(hh == KC - 1))
                          osb = p4sb.tile([128, 512], F32, tag="osb", name="osb")
                          if apply_bias_o:
                              bob = p4sb.tile([128, 512], F32, tag="bob",
                                              name="bob")
                              nc.gpsimd.partition_broadcast(
                                  bob[:, :], bvo_sb[1:2, c0:c0 + csz])
                              nc.vector.tensor_tensor(
                                  osb[0:tsz, :], ps[0:tsz, :], bob[0:tsz, :],
                                  mybir.AluOpType.add)
                          else:
                              nc.vector.tensor_copy(osb[0:tsz, :], ps[0:tsz, :])
                          nc.sync.dma_start(
                              out=out_part[t0:t0 + tsz, c0:c0 + csz],
                              in_=osb[0:tsz, :])

    nc.compile()
    return nc


_NC_CACHE = {}


def _get_nc(key):
    if key not in _NC_CACHE:
        _NC_CACHE[key] = build_kernel(*key)
    return _NC_CACHE[key]


def _prep_inputs(x, freqs_cos, freqs_sin, Wq, bq, Wk, bk, Wv, bv, Wo, bo,
                 gq, gk, frame_seqlen, debug=False):
    assert int(frame_seqlen) == L
    x2d = np.asarray(x, np.float32).reshape(T, D)
    xT_full = np.ascontiguousarray(x2d.T)

    perm = np.concatenate([
        np.concatenate([np.arange(0, 128, 2), np.arange(1, 128, 2)]) + 128 * h
        for h in range(H)])
    Wqp = np.asarray(Wq, np.float32)[:, perm]
    Wkp = np.asarray(Wk, np.float32)[:, perm]
    bqp = np.asarray(bq, np.float32)[perm]
    bkp = np.asarray(bk, np.float32)[perm]
    gqp = np.asarray(gq, np.float32)[perm]
    gkp = np.asarray(gk, np.float32)[perm]

    cosT = np.asarray(freqs_cos, np.float32).T
    sinT = np.asarray(freqs_sin, np.float32).T
    costab = np.concatenate([cosT, cosT], 0)
    sintab = np.concatenate([-sinT, sinT], 0)

    frames = np.arange(T) // L
    bf16 = ml_dtypes.bfloat16

    apply_bias_qk = not (np.all(bqp == 0) and np.all(bkp == 0))
    apply_g = not (np.all(gqp == 1) and np.all(gkp == 1))
    apply_bias_v = not np.all(np.asarray(bv) == 0)
    apply_bias_o = not np.all(np.asarray(bo) == 0)
    key = (apply_bias_qk, apply_g, apply_bias_v, apply_bias_o, debug)

    def tile_lhsT(w):  # [D, D] -> [KC, 128, KC*128]: out[d, p, c*128+m] = w[c*128+p, d*128+m]
        return np.ascontiguousarray(
            w.reshape(KC, 128, KC, 128).transpose(2, 1, 0, 3)
            .reshape(KC, 128, KC * 128))

    def tile_rhs(w):  # [D, D] -> [3, 128, KC*512]: out[g, p, c*512+m] = w[c*128+p, g*512+m]
        return np.ascontiguousarray(
            w.reshape(KC, 128, 3, 512).transpose(2, 1, 0, 3)
            .reshape(3, 128, KC * 512))

    shared = {
        "wq": tile_lhsT(Wqp).astype(bf16), "wk": tile_lhsT(Wkp).astype(bf16),
        "wv": tile_rhs(np.asarray(Wv, np.float32)).astype(bf16),
        "wo": tile_rhs(np.asarray(Wo, np.float32)).astype(bf16),
        "bqk2": np.concatenate([bqp, bkp]).reshape(2 * KC, 128),
        "gqk2": np.concatenate([gqp, gkp]).reshape(2 * KC, 128),
        "bvo": np.stack([np.asarray(bv, np.float32),
                         np.asarray(bo, np.float32)]),
    }
    in_maps = []
    for c in range(NC):
        t0 = c * CHUNK
        f_c = t0 // L
        rank_frames = (np.arange(NC) * CHUNK) // L
        mrank = np.where(rank_frames <= f_c, 0.0, -30000.0).astype(np.float32)
        mpad = np.broadcast_to(mrank, (128, NC)).copy()
        in_maps.append({
            **shared,
            "xT": np.ascontiguousarray(xT_full[:, t0:t0 + CHUNK]).astype(bf16),
            "cost": np.ascontiguousarray(costab[:, t0:t0 + CHUNK]),
            "sint": np.ascontiguousarray(sintab[:, t0:t0 + CHUNK]),
            "maskv": mpad,
        })
    return key, in_maps


def kernel(x, freqs_cos, freqs_sin, Wq, bq, Wk, bk, Wv, bv, Wo, bo,
           gq, gk, frame_seqlen):
    key, in_maps = _prep_inputs(x, freqs_cos, freqs_sin, Wq, bq, Wk, bk,
                                Wv, bv, Wo, bo, gq, gk, frame_seqlen)
    nc = _get_nc(key)
    res = run_bass_kernel_spmd(nc, in_maps, core_ids=list(range(NC)))
    out = np.empty((1, T, D), np.float32)
    for c in range(NC):
        out[0, c * CHUNK:(c + 1) * CHUNK, :] = res.results[c]["out_part"]
    return out

